# revision 1
# baseline (speedup 1.0000x reference)
"""Trainium2 Bass kernel for DiscreteDeltaThetaGammaLayer.

Coupled Kuramoto-oscillator recurrence:
  phase0 = (x @ W_phase.T) mod 2pi ; amp0 = max(|x @ W_amp.T|, eps)
  32 steps of: intra-band Kuramoto coupling (phase), PAC amplitude modulation
  output: final amp  (4096, 352) f32

Key structural facts exploited:
  - amp never feeds back into phase, K is block-diagonal, and the PAC
    modulation uses only delta/theta band means -> the 256 gamma phases
    never influence the output. Only the 96 delta+theta oscillators need
    the on-device recurrence; amp0 is needed for all 352.
  - K is uniform within each band, so the device only needs per-batch
    band sums (Sd,St,Cd,Ct) per step; the host reconstructs the exact
    clamped amp recurrence in closed form from those.

Device strategy (8 NeuronCores, data-parallel over batch, 512 rows/core):
  - Phase state chi = phi - pi/4 wrapped to [-pi, pi]; sin phi and cos phi
    are then BOTH direct ACT Sin calls (bias=pi/4, scale=+/-1) with args in
    [-1.25pi, 1.25pi] (Sin LUT error <= 2.5e-3 in the outer 12.5% tail).
    No Abs pass.
  - sin/cos written as one bf16 tile [cos | sin]; coupling = 2 bf16 matmuls
    per stream into PSUM [V|U]; mm = cs*vu one TT pass; d = m2-m1 (bf16 2x);
    chi' = WRAP_SUB(chi, d, dt*omega) custom DVE op.
  - Two batch streams (256 each) pipeline the ACT->PE->DVE/Pool chain;
    stream B's mm runs on the Pool engine to unload DVE.
  - Band sums are matmul'd directly into a PSUM stash (col = q*128 +
    step*4 + {Sd,St,Cd,Ct}) and DMA'd once at the end. amp0 f32r matmuls
    are hand-interleaved one-per-step into the PE's idle time and DMA'd
    straight from PSUM; |.| and the clamp run on the host.
"""

import math
import sys

sys.path.insert(0, "/opt/trn_rl_repo")

import numpy as np

# ---- problem constants (module hyperparameters) ----
N_DELTA, N_THETA, N_GAMMA = 32, 64, 256
N_TOTAL = 352
N_DIMS = 1024
BATCH = 4096
N_STEPS = 32
DT = 0.01
COUPLING = 2.0
PAC = 0.3
EPS = 1e-6
TWO_PI = 2.0 * math.pi
PI = math.pi

N_CORES = 8
BL = BATCH // N_CORES          # 512 batch rows per core
NS = 2                         # streams
BH = BL // NS                  # 256 batch per stream
ND = 96                        # delta+theta oscillators on device
P = 128
KD = N_DIMS // P               # 8 contraction chunks
NCH = 3                        # amp0 oscillator chunks (3*128 = 384 >= 352)

LAST_EXEC_NS = None
_COMPILED = {}
_WRAP_SUB = None


def _get_wrap_sub():
    """Custom DVE op: out = wrap((in0 - in1) + s0) into [-s1, s1], period imm2."""
    global _WRAP_SUB
    if _WRAP_SUB is not None:
        return _WRAP_SUB
    from concourse.dve_spec import C0, C1, C2, Spec, Src0, Src1, lower
    from concourse.dve_uop import DveOpSpec
    import concourse.dve_ops as dvo

    def _ref(in0, in1, s0, s1, imm2):
        y = (in0 - in1) + s0
        return (y + imm2 * ((y < -s1).astype(np.float32)
                            - (y > s1).astype(np.float32))).astype(np.float32)

    _y = (Src0 - Src1) + C0
    spec = Spec(body=_y + C2 * ((_y < -C1) - (_y > C1)), reference=_ref)
    shas = {}
    for ver in ("v3", "v4"):
        tmp = DveOpSpec(name="WRAP_SUB_KERNEL", opcode=31,
                        uops=lower(spec, ver=ver), rd1_en=True)
        shas[ver] = tmp.sha(ver)
    op = dvo.DveOp("WRAP_SUB_KERNEL", spec, subdim=False, uops_sha=shas)
    dvo.OPS.append(op)
    dvo.CUSTOM_DVE_SPECS[op.name] = op.spec
    dvo._SUB_OPCODE_FOR_NAME[op.name] = dvo._CUSTOM_DVE_ROW_BASE + len(dvo.OPS) - 1
    _WRAP_SUB = op
    return op


import contextlib


def _nullctx():
    return contextlib.nullcontext()


def _build_program(stagger_ns=0, upd_prio=0, pace_ns=0, pace_t0=6000):
    import concourse.bass as bass
    import concourse.tile as tile
    from concourse import bacc, mybir

    wrap_sub = _get_wrap_sub()

    f32 = mybir.dt.float32
    f32r = mybir.dt.float32r
    bf16 = mybir.dt.bfloat16
    AF = mybir.ActivationFunctionType
    ALU = mybir.AluOpType

    nc = bacc.Bacc("TRN2", target_bir_lowering=False, debug=False)

    # ---- DRAM I/O ----
    xT = nc.dram_tensor("xT", [N_DIMS, BL], f32r, kind="ExternalInput").ap()
    wpT = nc.dram_tensor("wpT", [N_DIMS, P], f32r, kind="ExternalInput").ap()
    waT = nc.dram_tensor("waT", [N_DIMS, NCH * P], f32r, kind="ExternalInput").ap()
    ktf = nc.dram_tensor("ktf", [P, P], f32, kind="ExternalInput").ap()
    wbandf = nc.dram_tensor("wbandf", [P, 2], f32, kind="ExternalInput").ap()
    dtw = nc.dram_tensor("dtw", [P, 1], f32, kind="ExternalInput").ap()

    amp0_out = nc.dram_tensor("amp0", [P, NCH * BL], f32, kind="ExternalOutput").ap()
    bs_out = nc.dram_tensor("bsums", [P, 4 * N_STEPS * 4], f32,
                            kind="ExternalOutput").ap()
    # bsums col = q*128 + step*4 + {Sd,St,Cd,Ct}; partition = batch q*128+p.

    with tile.TileContext(nc) as tc:
        with (
            tc.tile_pool(name="state", bufs=1) as state_pool,
            tc.tile_pool(name="weights", bufs=1) as wpool,
            tc.tile_pool(name="work", bufs=3) as work,
            tc.tile_pool(name="psum", bufs=1, space="PSUM") as psum,
        ):
            # ---- persistent constants (Pool DMA queue: cheap seq cost) ----
            dtw_sb = wpool.tile([P, 1], f32, tag="dtw", name="dtw_sb")
            nc.gpsimd.dma_start(dtw_sb[:], dtw[:])
            pi4 = wpool.tile([P, 1], f32, tag="pi4", name="pi4")
            nc.vector.memset(pi4[:], PI / 4.0)
            ktf_sb = wpool.tile([P, P], f32, tag="ktld", name="ktf_sb")
            nc.gpsimd.dma_start(ktf_sb[:], ktf[:])
            kt_sb = wpool.tile([P, P], bf16, tag="kt", name="kt_sb")
            nc.vector.tensor_copy(kt_sb[:], ktf_sb[:])
            wbf_sb = wpool.tile([P, 2], f32, tag="wbf", name="wbf_sb")
            nc.gpsimd.dma_start(wbf_sb[:], wbandf[:])
            wband_sb = wpool.tile([P, 2], bf16, tag="wband", name="wband_sb")
            nc.vector.tensor_copy(wband_sb[:], wbf_sb[:])

            # ---- input loads ----
            # phase path (wpT + x) split over sync/gpsimd queues; waT follows
            # on gpsimd and only gates the (lagging) amp matmuls.
            xk = []
            wpk = []
            wak = []
            for k in range(KD):
                t = wpool.tile([P, P], f32r, tag=f"wp_{k}", name=f"wp_{k}")
                nc.sync.dma_start(t[:], wpT[k * P:(k + 1) * P, :])
                wpk.append(t)
            for k in range(KD):
                t = wpool.tile([P, BL], f32r, tag=f"x_{k}", name=f"x_{k}")
                eng = nc.sync if k % 2 == 0 else nc.gpsimd
                eng.dma_start(t[:], xT[k * P:(k + 1) * P, :])
                xk.append(t)
            for k in range(KD):
                t = wpool.tile([P, NCH * P], f32r, tag=f"wa_{k}", name=f"wa_{k}")
                nc.gpsimd.dma_start(t[:], waT[k * P:(k + 1) * P, :])
                wak.append(t)

            # ---- PSUM tiles ----
            vu = [psum.tile([P, 2 * BH], f32, tag=f"vu{h}", name=f"vu{h}")
                  for h in range(NS)]
            stash = psum.tile([P, 4 * N_STEPS * 4], f32, tag="stash",
                              name="stash")
            amp_ps = [psum.tile([P, BL], f32, tag=f"amp{c}", name=f"amp{c}")
                      for c in range(NCH)]

            # ---- phase0 projection (f32r, 256-wide => full PE rate) ----
            phi = [state_pool.tile([P, BH], f32, tag=f"phi{h}", name=f"phi{h}")
                   for h in range(NS)]
            for h in range(NS):
                dst = vu[h][:, 0:BH]
                for k in range(KD):
                    nc.tensor.matmul(dst, wpk[k][:],
                                     xk[k][:, h * BH:(h + 1) * BH],
                                     start=(k == 0), stop=(k == KD - 1))
                # chi0 = wrap(phase0 - pi/4); stream B staggered ~half a
                # step-chain so the two streams' loops anti-phase interleave
                with tc.tile_wait_until(h * stagger_ns * 1e-6,
                                        enable=(stagger_ns > 0 and h > 0)):
                    nc.vector.add_range_wrap(phi[h][:], dst, -PI / 4.0, PI,
                                             TWO_PI)

            # ---- recurrence ----
            # amp0 matmuls are drip-fed into PE idle slots: one per (it, h).
            amp_jobs = [(c, k) for c in range(NCH) for k in range(KD)]
            job = 0

            cs_live = {}

            def trig_and_pe(h, it):
                """sin/cos + band sums + coupling matmuls for (h, it)."""
                nonlocal job
                ph = phi[h]
                cs = work.tile([P, 2 * BH], bf16, tag=f"cs{h}", name=f"cs{h}")
                cs_live[h] = cs
                cos = cs[:, 0:BH]
                sin = cs[:, BH:2 * BH]
                # sin(phi) = Sin(chi + pi/4) ; cos(phi) = Sin(-chi + pi/4)
                nc.scalar.activation(sin, ph[:], AF.Sin, bias=pi4[:],
                                     scale=1.0)
                nc.scalar.activation(cos, ph[:], AF.Sin, bias=pi4[:],
                                     scale=-1.0)
                if it < N_STEPS:
                    # coupling: vu = [V | U] = dtK @ [sin | cos]
                    nc.tensor.matmul(vu[h][:, 0:BH], kt_sb[:], sin,
                                     start=True, stop=True)
                    nc.tensor.matmul(vu[h][:, BH:2 * BH], kt_sb[:], cos,
                                     start=True, stop=True)
                # band sums -> stash[batch_part, q*128 + (it-1)*4 + j]
                if it > 0:
                    for q in range(BH // P):
                        qg = h * (BH // P) + q
                        base = qg * P + (it - 1) * 4
                        nc.tensor.matmul(
                            stash[:, base:base + 2],
                            sin[:, q * P:(q + 1) * P], wband_sb[:],
                            start=True, stop=True)
                        nc.tensor.matmul(
                            stash[:, base + 2:base + 4],
                            cos[:, q * P:(q + 1) * P], wband_sb[:],
                            start=True, stop=True)
                # one amp0 matmul per (it, h) PE-idle slot
                if it < N_STEPS and job < len(amp_jobs):
                    c, k = amp_jobs[job]
                    job += 1
                    nc.tensor.matmul(amp_ps[c][:],
                                     wak[k][:, c * P:(c + 1) * P],
                                     xk[k][:], start=(k == 0),
                                     stop=(k == KD - 1))
                    if k == KD - 1:
                        ab = work.tile([P, BL], f32, tag=f"ab{c}",
                                       name=f"ab{c}")
                        nc.scalar.copy(ab[:], amp_ps[c][:])
                        nc.sync.dma_start(
                            amp0_out[:, c * BL:(c + 1) * BL], ab[:])

            def update(h, prio=0):
                """mm + d + wrap for stream h's most recent trig."""
                ph = phi[h]
                cs = cs_live[h]
                with tc.high_priority(offset=prio) if prio else _nullctx():
                    # mm = [cos*V | sin*U] on DVE (Pool cannot read PSUM)
                    mm = work.tile([P, 2 * BH], bf16, tag=f"mm{h}",
                                   name=f"mm{h}")
                    nc.vector.tensor_tensor(mm[:], cs[:], vu[h][:], ALU.mult)
                    # d = sin*U - cos*V (bf16 2x, back-to-back on DVE)
                    d = work.tile([P, BH], bf16, tag=f"d{h}", name=f"d{h}")
                    nc.vector.tensor_tensor(d[:], mm[:, BH:2 * BH],
                                            mm[:, 0:BH], ALU.subtract)
                    # chi' = wrap((chi - d) + dt*omega)
                    nc.vector._custom_dve(wrap_sub, out=ph[:], in0=ph[:],
                                          in1=d[:], s0=dtw_sb[:], s1=PI,
                                          imm2=TWO_PI)

            # Software-pipelined anti-phase: stream B runs half a step
            # behind A, so B's DVE burst fills A's trig/PE window. The
            # tile_wait_until hints pace the scheduler's model (the runtime
            # ignores them) so the static order interleaves anti-phase.
            def slot(ns):
                return tc.tile_wait_until(ns * 1e-6, enable=pace_ns > 0)

            for it in range(N_STEPS + 1):
                t0 = pace_t0 + it * pace_ns
                with slot(t0):
                    trig_and_pe(0, it)
                if it > 0:
                    with slot(t0):
                        update(1, prio=upd_prio)   # B's step it-1
                with slot(t0 + pace_ns // 2):
                    trig_and_pe(1, it)
                if it < N_STEPS:
                    with slot(t0 + pace_ns // 2):
                        update(0, prio=upd_prio)   # A's step it

            # flush remaining amp jobs (if any) and the stash
            while job < len(amp_jobs):
                c, k = amp_jobs[job]
                job += 1
                nc.tensor.matmul(amp_ps[c][:], wak[k][:, c * P:(c + 1) * P],
                                 xk[k][:], start=(k == 0), stop=(k == KD - 1))
                if k == KD - 1:
                    ab = work.tile([P, BL], f32, tag=f"ab{c}", name=f"ab{c}")
                    nc.scalar.copy(ab[:], amp_ps[c][:])
                    nc.sync.dma_start(amp0_out[:, c * BL:(c + 1) * BL], ab[:])
            st_sb = work.tile([P, 4 * N_STEPS * 4], f32, tag="st_sb",
                              name="st_sb")
            nc.scalar.copy(st_sb[:], stash[:])
            nc.sync.dma_start(bs_out[:], st_sb[:])

    nc.compile()
    return nc


def kernel(x, W_phase, W_amp, omega, K):
    from concourse.bass_utils import run_bass_kernel_spmd

    x = np.asarray(x, dtype=np.float32)
    W_phase = np.asarray(W_phase, dtype=np.float32)
    W_amp = np.asarray(W_amp, dtype=np.float32)
    omega = np.asarray(omega, dtype=np.float32)
    K = np.asarray(K, dtype=np.float32)

    # ---- host-side packing ----
    wpT = np.zeros((N_DIMS, P), dtype=np.float32)
    wpT[:, :ND] = W_phase[:ND].T
    waT = np.zeros((N_DIMS, NCH * P), dtype=np.float32)
    for c in range(NCH):
        n = min(P, N_TOTAL - c * P)
        waT[:, c * P:c * P + n] = W_amp[c * P:c * P + n].T

    ktf = np.zeros((P, P), dtype=np.float32)
    ktf[:ND, :ND] = DT * K[:ND, :ND].T

    wband = np.zeros((P, 2), dtype=np.float32)
    wband[:N_DELTA, 0] = 1.0
    wband[N_DELTA:ND, 1] = 1.0

    dtw = np.zeros((P, 1), dtype=np.float32)
    w = DT * omega[:ND].astype(np.float64)
    dtw[:ND, 0] = (np.mod(w + PI, TWO_PI) - PI).astype(np.float32)

    if "prog" not in _COMPILED:
        _COMPILED["prog"] = _build_program()
    nc = _COMPILED["prog"]

    in_maps = []
    for i in range(N_CORES):
        xst = np.ascontiguousarray(x[i * BL:(i + 1) * BL].T)
        in_maps.append({
            "xT": xst, "wpT": wpT, "waT": waT, "ktf": ktf,
            "wbandf": wband, "dtw": dtw,
        })

    res = run_bass_kernel_spmd(nc, in_maps, core_ids=list(range(N_CORES)))

    # ---- host-side unshard + exact amp reconstruction ----
    band_of = np.zeros(N_TOTAL, dtype=np.int64)
    band_of[N_DELTA:ND] = 1
    band_of[ND:] = 2

    out = np.empty((BATCH, N_TOTAL), dtype=np.float32)
    for i in range(N_CORES):
        r = res.results[i]
        a0 = np.empty((BL, N_TOTAL))
        raw = r["amp0"].astype(np.float64)          # [128, 3*512]
        for c in range(NCH):
            n = min(P, N_TOTAL - c * P)
            a0[:, c * P:c * P + n] = raw[:n, c * BL:(c + 1) * BL].T
        a0 = np.maximum(np.abs(a0), EPS)

        bs = r["bsums"].astype(np.float64).reshape(P, 4, N_STEPS, 4)
        # [p, q, k, j] -> batch b = q*128+p
        S = np.empty((BL, N_STEPS, 2))
        C = np.empty((BL, N_STEPS, 2))
        for q in range(4):
            sl = slice(q * P, (q + 1) * P)
            S[sl] = bs[:, q, :, 0:2]
            C[sl] = bs[:, q, :, 2:4]
        cosm = C / np.sqrt(S * S + C * C)           # [b, k, band(d,t)]
        f = 1.0 + DT * PAC * cosm
        Pk = np.cumprod(f, axis=1)
        mk = np.minimum.accumulate(Pk, axis=1)
        Pn = Pk[:, -1]                              # [b, 2]
        mn = mk[:, -1]
        Pfac = np.ones((BL, 3))
        Efac = np.ones((BL, 3))
        Pfac[:, 1] = Pn[:, 0]
        Pfac[:, 2] = Pn[:, 1]
        Efac[:, 1] = Pn[:, 0] / mn[:, 0]
        Efac[:, 2] = Pn[:, 1] / mn[:, 1]
        amp = np.maximum(a0 * Pfac[:, band_of], EPS * Efac[:, band_of])
        out[i * BL:(i + 1) * BL] = amp.astype(np.float32)
    return out



# revision 2
# speedup vs baseline: 1.0322x; 1.0322x over previous
"""Trainium2 Bass kernel for DiscreteDeltaThetaGammaLayer.

Coupled Kuramoto-oscillator recurrence:
  phase0 = (x @ W_phase.T) mod 2pi ; amp0 = max(|x @ W_amp.T|, eps)
  32 steps of: intra-band Kuramoto coupling (phase), PAC amplitude modulation
  output: final amp  (4096, 352) f32

Key structural facts exploited:
  - amp never feeds back into phase, K is block-diagonal, and the PAC
    modulation uses only delta/theta band means -> the 256 gamma phases
    never influence the output. Only the 96 delta+theta oscillators need
    the on-device recurrence; amp0 is needed for all 352.
  - K is uniform within each band, so the device only needs per-batch
    band sums (Sd,St,Cd,Ct) per step; the host reconstructs the exact
    clamped amp recurrence in closed form from those.

Device strategy (8 NeuronCores, data-parallel over batch, 512 rows/core):
  - Phase state chi = phi - pi/4 wrapped to [-pi, pi]; sin phi and cos phi
    are then BOTH direct ACT Sin calls (bias=pi/4, scale=+/-1) with args in
    [-1.25pi, 1.25pi] (Sin LUT error <= 2.5e-3 in the outer 12.5% tail).
    No Abs pass.
  - sin/cos written as one bf16 tile [cos | sin]; coupling = 2 bf16 matmuls
    per stream into PSUM [V|U]; mm = cs*vu one TT pass; d = m2-m1 (bf16 2x);
    chi' = WRAP_SUB(chi, d, dt*omega) custom DVE op.
  - Two batch streams (256 each) pipeline the ACT->PE->DVE/Pool chain;
    stream B's mm runs on the Pool engine to unload DVE.
  - Band sums are matmul'd directly into a PSUM stash (col = q*128 +
    step*4 + {Sd,St,Cd,Ct}) and DMA'd once at the end. amp0 f32r matmuls
    are hand-interleaved one-per-step into the PE's idle time and DMA'd
    straight from PSUM; |.| and the clamp run on the host.
"""

import math
import sys

sys.path.insert(0, "/opt/trn_rl_repo")

import numpy as np

# ---- problem constants (module hyperparameters) ----
N_DELTA, N_THETA, N_GAMMA = 32, 64, 256
N_TOTAL = 352
N_DIMS = 1024
BATCH = 4096
N_STEPS = 32
DT = 0.01
COUPLING = 2.0
PAC = 0.3
EPS = 1e-6
TWO_PI = 2.0 * math.pi
PI = math.pi

N_CORES = 8
BL = BATCH // N_CORES          # 512 batch rows per core
NS = 2                         # streams
BH = BL // NS                  # 256 batch per stream
ND = 96                        # delta+theta oscillators on device
P = 128
KD = N_DIMS // P               # 8 contraction chunks
NCH = 3                        # amp0 oscillator chunks (3*128 = 384 >= 352)

LAST_EXEC_NS = None
_COMPILED = {}
_WRAP_SUB = None


def _get_wrap_sub():
    """Custom DVE op: out = wrap((in0 - in1) + s0) into [-s1, s1], period imm2."""
    global _WRAP_SUB
    if _WRAP_SUB is not None:
        return _WRAP_SUB
    from concourse.dve_spec import C0, C1, C2, Spec, Src0, Src1, lower
    from concourse.dve_uop import DveOpSpec
    import concourse.dve_ops as dvo

    def _ref(in0, in1, s0, s1, imm2):
        y = (in0 - in1) + s0
        return (y + imm2 * ((y < -s1).astype(np.float32)
                            - (y > s1).astype(np.float32))).astype(np.float32)

    _y = (Src0 - Src1) + C0
    spec = Spec(body=_y + C2 * ((_y < -C1) - (_y > C1)), reference=_ref)
    shas = {}
    for ver in ("v3", "v4"):
        tmp = DveOpSpec(name="WRAP_SUB_KERNEL", opcode=31,
                        uops=lower(spec, ver=ver), rd1_en=True)
        shas[ver] = tmp.sha(ver)
    op = dvo.DveOp("WRAP_SUB_KERNEL", spec, subdim=False, uops_sha=shas)
    dvo.OPS.append(op)
    dvo.CUSTOM_DVE_SPECS[op.name] = op.spec
    dvo._SUB_OPCODE_FOR_NAME[op.name] = dvo._CUSTOM_DVE_ROW_BASE + len(dvo.OPS) - 1
    _WRAP_SUB = op
    return op


import contextlib


def _nullctx():
    return contextlib.nullcontext()


def _build_program(stagger_ns=0, upd_prio=0, pace_ns=0, pace_t0=6000):
    import concourse.bass as bass
    import concourse.tile as tile
    from concourse import bacc, mybir

    wrap_sub = _get_wrap_sub()

    f32 = mybir.dt.float32
    f32r = mybir.dt.float32r
    bf16 = mybir.dt.bfloat16
    AF = mybir.ActivationFunctionType
    ALU = mybir.AluOpType

    nc = bacc.Bacc("TRN2", target_bir_lowering=False, debug=False)

    # ---- DRAM I/O ----
    xT = nc.dram_tensor("xT", [N_DIMS, BL], f32r, kind="ExternalInput").ap()
    wpT = nc.dram_tensor("wpT", [N_DIMS, P], f32r, kind="ExternalInput").ap()
    waT = nc.dram_tensor("waT", [N_DIMS, NCH * P], f32r, kind="ExternalInput").ap()
    ktf = nc.dram_tensor("ktf", [P, P], f32, kind="ExternalInput").ap()
    wbandf = nc.dram_tensor("wbandf", [P, 2], f32, kind="ExternalInput").ap()
    dtw = nc.dram_tensor("dtw", [P, 1], f32, kind="ExternalInput").ap()

    amp0_out = nc.dram_tensor("amp0", [P, NCH * BL], f32, kind="ExternalOutput").ap()
    bs_out = nc.dram_tensor("bsums", [P, 4 * N_STEPS * 4], f32,
                            kind="ExternalOutput").ap()
    # bsums col = q*128 + step*4 + {Sd,St,Cd,Ct}; partition = batch q*128+p.

    with tile.TileContext(nc) as tc:
        with (
            tc.tile_pool(name="state", bufs=1) as state_pool,
            tc.tile_pool(name="weights", bufs=1) as wpool,
            tc.tile_pool(name="work", bufs=3) as work,
            tc.tile_pool(name="psum", bufs=1, space="PSUM") as psum,
        ):
            # ---- persistent constants (Pool DMA queue: cheap seq cost) ----
            dtw_sb = wpool.tile([P, 1], f32, tag="dtw", name="dtw_sb")
            nc.gpsimd.dma_start(dtw_sb[:], dtw[:])
            pi4 = wpool.tile([P, 1], f32, tag="pi4", name="pi4")
            nc.vector.memset(pi4[:], PI / 4.0)
            ktf_sb = wpool.tile([P, P], f32, tag="ktld", name="ktf_sb")
            nc.gpsimd.dma_start(ktf_sb[:], ktf[:])
            kt_sb = wpool.tile([P, P], bf16, tag="kt", name="kt_sb")
            nc.vector.tensor_copy(kt_sb[:], ktf_sb[:])
            wbf_sb = wpool.tile([P, 2], f32, tag="wbf", name="wbf_sb")
            nc.gpsimd.dma_start(wbf_sb[:], wbandf[:])
            wband_sb = wpool.tile([P, 2], bf16, tag="wband", name="wband_sb")
            nc.vector.tensor_copy(wband_sb[:], wbf_sb[:])

            # ---- input loads ----
            # phase path (wpT + x) split over sync/gpsimd queues; waT follows
            # on gpsimd and only gates the (lagging) amp matmuls.
            xk = []
            wpk = []
            wak = []
            for k in range(KD):
                t = wpool.tile([P, P], f32r, tag=f"wp_{k}", name=f"wp_{k}")
                nc.sync.dma_start(t[:], wpT[k * P:(k + 1) * P, :])
                wpk.append(t)
            for k in range(KD):
                t = wpool.tile([P, BL], f32r, tag=f"x_{k}", name=f"x_{k}")
                eng = nc.sync if k % 2 == 0 else nc.gpsimd
                eng.dma_start(t[:], xT[k * P:(k + 1) * P, :])
                xk.append(t)
            for k in range(KD):
                t = wpool.tile([P, NCH * P], f32r, tag=f"wa_{k}", name=f"wa_{k}")
                nc.gpsimd.dma_start(t[:], waT[k * P:(k + 1) * P, :])
                wak.append(t)

            # ---- PSUM tiles ----
            vu = [psum.tile([P, 2 * BH], f32, tag=f"vu{h}", name=f"vu{h}")
                  for h in range(NS)]
            stash = psum.tile([P, 4 * N_STEPS * 4], f32, tag="stash",
                              name="stash")
            amp_ps = [psum.tile([P, BL], f32, tag=f"amp{c}", name=f"amp{c}")
                      for c in range(NCH)]

            # ---- phase0 projection (f32r, 256-wide => full PE rate) ----
            phi = [state_pool.tile([P, BH], f32, tag=f"phi{h}", name=f"phi{h}")
                   for h in range(NS)]
            for h in range(NS):
                dst = vu[h][:, 0:BH]
                for k in range(KD):
                    nc.tensor.matmul(dst, wpk[k][:],
                                     xk[k][:, h * BH:(h + 1) * BH],
                                     start=(k == 0), stop=(k == KD - 1))
                # chi0 = wrap(phase0 - pi/4); stream B staggered ~half a
                # step-chain so the two streams' loops anti-phase interleave
                with tc.tile_wait_until(h * stagger_ns * 1e-6,
                                        enable=(stagger_ns > 0 and h > 0)):
                    nc.vector.add_range_wrap(phi[h][:], dst, -PI / 4.0, PI,
                                             TWO_PI)

            # ---- recurrence ----
            # amp0 matmuls are drip-fed into PE idle slots: one per (it, h).
            amp_jobs = [(c, k) for c in range(NCH) for k in range(KD)]
            job = 0

            cs_live = {}

            def trig_and_pe(h, it):
                """sin/cos + band sums + coupling matmuls for (h, it)."""
                nonlocal job
                ph = phi[h]
                cs = work.tile([P, 2 * BH], bf16, tag=f"cs{h}", name=f"cs{h}")
                cs_live[h] = cs
                cos = cs[:, 0:BH]
                sin = cs[:, BH:2 * BH]
                # sin(phi) = Sin(chi + pi/4) ; cos(phi) = Sin(-chi + pi/4)
                nc.scalar.activation(sin, ph[:], AF.Sin, bias=pi4[:],
                                     scale=1.0)
                nc.scalar.activation(cos, ph[:], AF.Sin, bias=pi4[:],
                                     scale=-1.0)
                if it < N_STEPS:
                    # coupling: vu = [V | U] = dtK @ [sin | cos]
                    nc.tensor.matmul(vu[h][:, 0:BH], kt_sb[:], sin,
                                     start=True, stop=True)
                    nc.tensor.matmul(vu[h][:, BH:2 * BH], kt_sb[:], cos,
                                     start=True, stop=True)
                # band sums -> stash[batch_part, q*128 + (it-1)*4 + j]
                if it > 0:
                    for q in range(BH // P):
                        qg = h * (BH // P) + q
                        base = qg * P + (it - 1) * 4
                        nc.tensor.matmul(
                            stash[:, base:base + 2],
                            sin[:, q * P:(q + 1) * P], wband_sb[:],
                            start=True, stop=True)
                        nc.tensor.matmul(
                            stash[:, base + 2:base + 4],
                            cos[:, q * P:(q + 1) * P], wband_sb[:],
                            start=True, stop=True)
                # one amp0 matmul per (it, h) PE-idle slot
                if it < N_STEPS and job < len(amp_jobs):
                    c, k = amp_jobs[job]
                    job += 1
                    nc.tensor.matmul(amp_ps[c][:],
                                     wak[k][:, c * P:(c + 1) * P],
                                     xk[k][:], start=(k == 0),
                                     stop=(k == KD - 1))
                    if k == KD - 1:
                        ab = work.tile([P, BL], f32, tag=f"ab{c}",
                                       name=f"ab{c}")
                        nc.scalar.copy(ab[:], amp_ps[c][:])
                        nc.sync.dma_start(
                            amp0_out[:, c * BL:(c + 1) * BL], ab[:])

            def update(h, prio=0):
                """mm + d + wrap for stream h's most recent trig."""
                ph = phi[h]
                cs = cs_live[h]
                with tc.high_priority(offset=prio) if prio else _nullctx():
                    # mm = [cos*V | sin*U] on DVE (Pool cannot read PSUM)
                    mm = work.tile([P, 2 * BH], bf16, tag=f"mm{h}",
                                   name=f"mm{h}")
                    nc.vector.tensor_tensor(mm[:], cs[:], vu[h][:], ALU.mult)
                    # d = sin*U - cos*V on Pool (GPSIMD): unloads DVE so the
                    # other stream's mm can run in this slot
                    d = work.tile([P, BH], bf16, tag=f"d{h}", name=f"d{h}")
                    nc.gpsimd.tensor_tensor(d[:], mm[:, BH:2 * BH],
                                            mm[:, 0:BH], ALU.subtract)
                    # chi' = wrap((chi - d) + dt*omega)
                    nc.vector._custom_dve(wrap_sub, out=ph[:], in0=ph[:],
                                          in1=d[:], s0=dtw_sb[:], s1=PI,
                                          imm2=TWO_PI)

            # Software-pipelined anti-phase: stream B runs half a step
            # behind A, so B's DVE burst fills A's trig/PE window. The
            # tile_wait_until hints pace the scheduler's model (the runtime
            # ignores them) so the static order interleaves anti-phase.
            def slot(ns):
                return tc.tile_wait_until(ns * 1e-6, enable=pace_ns > 0)

            for it in range(N_STEPS + 1):
                t0 = pace_t0 + it * pace_ns
                with slot(t0):
                    trig_and_pe(0, it)
                if it > 0:
                    with slot(t0):
                        update(1, prio=upd_prio)   # B's step it-1
                with slot(t0 + pace_ns // 2):
                    trig_and_pe(1, it)
                if it < N_STEPS:
                    with slot(t0 + pace_ns // 2):
                        update(0, prio=upd_prio)   # A's step it

            # flush remaining amp jobs (if any) and the stash
            while job < len(amp_jobs):
                c, k = amp_jobs[job]
                job += 1
                nc.tensor.matmul(amp_ps[c][:], wak[k][:, c * P:(c + 1) * P],
                                 xk[k][:], start=(k == 0), stop=(k == KD - 1))
                if k == KD - 1:
                    ab = work.tile([P, BL], f32, tag=f"ab{c}", name=f"ab{c}")
                    nc.scalar.copy(ab[:], amp_ps[c][:])
                    nc.sync.dma_start(amp0_out[:, c * BL:(c + 1) * BL], ab[:])
            st_sb = work.tile([P, 4 * N_STEPS * 4], f32, tag="st_sb",
                              name="st_sb")
            nc.scalar.copy(st_sb[:], stash[:])
            nc.sync.dma_start(bs_out[:], st_sb[:])

    nc.compile()
    return nc


def kernel(x, W_phase, W_amp, omega, K):
    from concourse.bass_utils import run_bass_kernel_spmd

    x = np.asarray(x, dtype=np.float32)
    W_phase = np.asarray(W_phase, dtype=np.float32)
    W_amp = np.asarray(W_amp, dtype=np.float32)
    omega = np.asarray(omega, dtype=np.float32)
    K = np.asarray(K, dtype=np.float32)

    # ---- host-side packing ----
    wpT = np.zeros((N_DIMS, P), dtype=np.float32)
    wpT[:, :ND] = W_phase[:ND].T
    waT = np.zeros((N_DIMS, NCH * P), dtype=np.float32)
    for c in range(NCH):
        n = min(P, N_TOTAL - c * P)
        waT[:, c * P:c * P + n] = W_amp[c * P:c * P + n].T

    ktf = np.zeros((P, P), dtype=np.float32)
    ktf[:ND, :ND] = DT * K[:ND, :ND].T

    wband = np.zeros((P, 2), dtype=np.float32)
    wband[:N_DELTA, 0] = 1.0
    wband[N_DELTA:ND, 1] = 1.0

    dtw = np.zeros((P, 1), dtype=np.float32)
    w = DT * omega[:ND].astype(np.float64)
    dtw[:ND, 0] = (np.mod(w + PI, TWO_PI) - PI).astype(np.float32)

    if "prog" not in _COMPILED:
        _COMPILED["prog"] = _build_program()
    nc = _COMPILED["prog"]

    in_maps = []
    for i in range(N_CORES):
        xst = np.ascontiguousarray(x[i * BL:(i + 1) * BL].T)
        in_maps.append({
            "xT": xst, "wpT": wpT, "waT": waT, "ktf": ktf,
            "wbandf": wband, "dtw": dtw,
        })

    res = run_bass_kernel_spmd(nc, in_maps, core_ids=list(range(N_CORES)))

    # ---- host-side unshard + exact amp reconstruction ----
    band_of = np.zeros(N_TOTAL, dtype=np.int64)
    band_of[N_DELTA:ND] = 1
    band_of[ND:] = 2

    out = np.empty((BATCH, N_TOTAL), dtype=np.float32)
    for i in range(N_CORES):
        r = res.results[i]
        a0 = np.empty((BL, N_TOTAL))
        raw = r["amp0"].astype(np.float64)          # [128, 3*512]
        for c in range(NCH):
            n = min(P, N_TOTAL - c * P)
            a0[:, c * P:c * P + n] = raw[:n, c * BL:(c + 1) * BL].T
        a0 = np.maximum(np.abs(a0), EPS)

        bs = r["bsums"].astype(np.float64).reshape(P, 4, N_STEPS, 4)
        # [p, q, k, j] -> batch b = q*128+p
        S = np.empty((BL, N_STEPS, 2))
        C = np.empty((BL, N_STEPS, 2))
        for q in range(4):
            sl = slice(q * P, (q + 1) * P)
            S[sl] = bs[:, q, :, 0:2]
            C[sl] = bs[:, q, :, 2:4]
        cosm = C / np.sqrt(S * S + C * C)           # [b, k, band(d,t)]
        f = 1.0 + DT * PAC * cosm
        Pk = np.cumprod(f, axis=1)
        mk = np.minimum.accumulate(Pk, axis=1)
        Pn = Pk[:, -1]                              # [b, 2]
        mn = mk[:, -1]
        Pfac = np.ones((BL, 3))
        Efac = np.ones((BL, 3))
        Pfac[:, 1] = Pn[:, 0]
        Pfac[:, 2] = Pn[:, 1]
        Efac[:, 1] = Pn[:, 0] / mn[:, 0]
        Efac[:, 2] = Pn[:, 1] / mn[:, 1]
        amp = np.maximum(a0 * Pfac[:, band_of], EPS * Efac[:, band_of])
        out[i * BL:(i + 1) * BL] = amp.astype(np.float32)
    return out



# revision 9
# speedup vs baseline: 1.0644x; 1.0312x over previous
"""Trainium2 Bass kernel for DiscreteDeltaThetaGammaLayer.

Coupled Kuramoto-oscillator recurrence:
  phase0 = (x @ W_phase.T) mod 2pi ; amp0 = max(|x @ W_amp.T|, eps)
  32 steps of: intra-band Kuramoto coupling (phase), PAC amplitude modulation
  output: final amp  (4096, 352) f32

Key structural facts exploited:
  - amp never feeds back into phase, K is block-diagonal, and the PAC
    modulation uses only delta/theta band means -> the 256 gamma phases
    never influence the output. Only the 96 delta+theta oscillators need
    the on-device recurrence; amp0 is needed for all 352.
  - K is uniform within each band, so the device only needs per-batch
    band sums (Sd,St,Cd,Ct) per step; the host reconstructs the exact
    clamped amp recurrence in closed form from those.

Device strategy (8 NeuronCores, data-parallel over batch, 512 rows/core):
  - Phase state chi = phi - pi/4 wrapped to [-pi, pi]; sin phi and cos phi
    are then BOTH direct ACT Sin calls (bias=pi/4, scale=+/-1) with args in
    [-1.25pi, 1.25pi] (Sin LUT error <= 2.5e-3 in the outer 12.5% tail).
    No Abs pass.
  - sin/cos written as one bf16 tile [cos | sin]; coupling = 2 bf16 matmuls
    per stream into PSUM [V|U]; mm = cs*vu one TT pass; d = m2-m1 (bf16 2x);
    chi' = WRAP_SUB(chi, d, dt*omega) custom DVE op.
  - Two batch streams (256 each) pipeline the ACT->PE->DVE/Pool chain;
    stream B's mm runs on the Pool engine to unload DVE.
  - Band sums are matmul'd directly into a PSUM stash (col = q*128 +
    step*4 + {Sd,St,Cd,Ct}) and DMA'd once at the end. amp0 f32r matmuls
    are hand-interleaved one-per-step into the PE's idle time and DMA'd
    straight from PSUM; |.| and the clamp run on the host.
"""

import math
import sys

sys.path.insert(0, "/opt/trn_rl_repo")

import numpy as np

# ---- problem constants (module hyperparameters) ----
N_DELTA, N_THETA, N_GAMMA = 32, 64, 256
N_TOTAL = 352
N_DIMS = 1024
BATCH = 4096
N_STEPS = 32
DT = 0.01
COUPLING = 2.0
PAC = 0.3
EPS = 1e-6
TWO_PI = 2.0 * math.pi
PI = math.pi

N_CORES = 8
BL = BATCH // N_CORES          # 512 batch rows per core
NS = 2                         # streams
BH = BL // NS                  # 256 batch per stream
ND = 96                        # delta+theta oscillators on device
P = 128
KD = N_DIMS // P               # 8 contraction chunks
NCH = 3                        # amp0 oscillator chunks (3*128 = 384 >= 352)

LAST_EXEC_NS = None
_COMPILED = {}
_WRAP_SUB = None


def _get_wrap_sub():
    """Custom DVE op: out = wrap((in0 - in1) + s0) into [-s1, s1], period imm2."""
    global _WRAP_SUB
    if _WRAP_SUB is not None:
        return _WRAP_SUB
    from concourse.dve_spec import C0, C1, C2, Spec, Src0, Src1, lower
    from concourse.dve_uop import DveOpSpec
    import concourse.dve_ops as dvo

    def _ref(in0, in1, s0, s1, imm2):
        y = (in0 - in1) + s0
        return (y + imm2 * ((y < -s1).astype(np.float32)
                            - (y > s1).astype(np.float32))).astype(np.float32)

    _y = (Src0 - Src1) + C0
    spec = Spec(body=_y + C2 * ((_y < -C1) - (_y > C1)), reference=_ref)
    shas = {}
    for ver in ("v3", "v4"):
        tmp = DveOpSpec(name="WRAP_SUB_KERNEL", opcode=31,
                        uops=lower(spec, ver=ver), rd1_en=True)
        shas[ver] = tmp.sha(ver)
    op = dvo.DveOp("WRAP_SUB_KERNEL", spec, subdim=False, uops_sha=shas)
    dvo.OPS.append(op)
    dvo.CUSTOM_DVE_SPECS[op.name] = op.spec
    dvo._SUB_OPCODE_FOR_NAME[op.name] = dvo._CUSTOM_DVE_ROW_BASE + len(dvo.OPS) - 1
    _WRAP_SUB = op
    return op


import contextlib


def _nullctx():
    return contextlib.nullcontext()


def _build_program(stagger_ns=0, upd_prio=0, pace_ns=0, pace_t0=6000):
    import concourse.bass as bass
    import concourse.tile as tile
    from concourse import bacc, mybir

    wrap_sub = _get_wrap_sub()

    f32 = mybir.dt.float32
    f32r = mybir.dt.float32r
    bf16 = mybir.dt.bfloat16
    AF = mybir.ActivationFunctionType
    ALU = mybir.AluOpType

    nc = bacc.Bacc("TRN2", target_bir_lowering=False, debug=False)

    # ---- DRAM I/O ----
    # bf16 inputs, host-packed so partition p's row is contiguous:
    #   xT  [P, KD*BL]  col k*BL+b  = x[b, k*128+p]
    #   wpT [P, KD*P]   col k*P+i   = W_phase[i, k*128+p] (i < ND)
    #   waT [P, KD*NCH*P] col k*NCH*P+c*P+i = W_amp[c*128+i, k*128+p]
    # consts [P, P+3] = [dt*K.T | wband(2) | dtw]
    xT = nc.dram_tensor("xT", [P, KD * BL], bf16, kind="ExternalInput").ap()
    wpT = nc.dram_tensor("wpT", [P, KD * P], bf16, kind="ExternalInput").ap()
    waT = nc.dram_tensor("waT", [P, KD * NCH * P], bf16,
                         kind="ExternalInput").ap()
    consts = nc.dram_tensor("consts", [P, P + 3], f32, kind="ExternalInput").ap()

    amp0_out = nc.dram_tensor("amp0", [P, NCH * BL], f32, kind="ExternalOutput").ap()
    bs_out = nc.dram_tensor("bsums", [P, 4 * N_STEPS * 4], f32,
                            kind="ExternalOutput").ap()
    # bsums col = q*128 + step*4 + {Sd,St,Cd,Ct}; partition = batch q*128+p.

    with tile.TileContext(nc) as tc:
        with (
            tc.tile_pool(name="state", bufs=1) as state_pool,
            tc.tile_pool(name="weights", bufs=1) as wpool,
            tc.tile_pool(name="work", bufs=3) as work,
            tc.tile_pool(name="psum", bufs=1, space="PSUM") as psum,
        ):
    # ---- persistent constants + big packed input loads ----
            cst_sb = wpool.tile([P, P + 3], f32, tag="cst", name="cst_sb")
            nc.sync.dma_start(cst_sb[:], consts[:])
            dtw_sb = cst_sb[:, P + 2:P + 3]
            pi4 = wpool.tile([P, 1], f32, tag="pi4", name="pi4")
            nc.vector.memset(pi4[:], PI / 4.0)
            kt_sb = wpool.tile([P, P], bf16, tag="kt", name="kt_sb")
            nc.vector.tensor_copy(kt_sb[:], cst_sb[:, 0:P])
            wband_sb = wpool.tile([P, 2], bf16, tag="wband", name="wband_sb")
            nc.vector.tensor_copy(wband_sb[:], cst_sb[:, P:P + 2])

            # big packed loads: wp first (proj stationaries), then x in two
            # halves (proj k-chunks start as each half lands), wa last on the
            # gpsimd queue (only gates the lagging amp matmuls).
            wp_all = wpool.tile([P, KD * P], bf16, tag="wp", name="wp_all")
            nc.sync.dma_start(wp_all[:], wpT[:])
            x_all = wpool.tile([P, KD * BL], bf16, tag="x", name="x_all")
            HK = KD // 2
            nc.sync.dma_start(x_all[:, 0:HK * BL], xT[:, 0:HK * BL])
            nc.sync.dma_start(x_all[:, HK * BL:], xT[:, HK * BL:])
            wa_all = wpool.tile([P, KD * NCH * P], bf16, tag="wa",
                                name="wa_all")
            nc.gpsimd.dma_start(wa_all[:], waT[:])
            wpk = [wp_all[:, k * P:(k + 1) * P] for k in range(KD)]
            xk = [x_all[:, k * BL:(k + 1) * BL] for k in range(KD)]
            wak = [wa_all[:, k * NCH * P:(k + 1) * NCH * P] for k in range(KD)]

            # ---- PSUM tiles ----
            vu = [psum.tile([P, 2 * BH], f32, tag=f"vu{h}", name=f"vu{h}")
                  for h in range(NS)]
            stash = psum.tile([P, 4 * N_STEPS * 4], f32, tag="stash",
                              name="stash")
            amp_ps = [psum.tile([P, BL], f32, tag=f"amp{c}", name=f"amp{c}")
                      for c in range(NCH)]

            # ---- phase0 projection (f32r, 256-wide => full PE rate) ----
            phi = [state_pool.tile([P, BH], f32, tag=f"phi{h}", name=f"phi{h}")
                   for h in range(NS)]
            for h in range(NS):
                dst = vu[h][:, 0:BH]
                for k in range(KD):
                    nc.tensor.matmul(dst, wpk[k],
                                     x_all[:, k * BL + h * BH:
                                           k * BL + (h + 1) * BH],
                                     start=(k == 0), stop=(k == KD - 1))
                # chi0 = wrap(phase0 - pi/4); stream B staggered ~half a
                # step-chain so the two streams' loops anti-phase interleave
                with tc.tile_wait_until(h * stagger_ns * 1e-6,
                                        enable=(stagger_ns > 0 and h > 0)):
                    nc.vector.add_range_wrap(phi[h][:], dst, -PI / 4.0, PI,
                                             TWO_PI)

            # ---- recurrence ----
            # amp0 matmuls are drip-fed into PE idle slots: one per (it, h).
            amp_jobs = [(c, k) for c in range(NCH) for k in range(KD)]
            job = 0

            cs_live = {}

            def trig_and_pe(h, it):
                """sin/cos + band sums + coupling matmuls for (h, it)."""
                nonlocal job
                ph = phi[h]
                cs = work.tile([P, 2 * BH], bf16, tag=f"cs{h}", name=f"cs{h}")
                cs_live[h] = cs
                cos = cs[:, 0:BH]
                sin = cs[:, BH:2 * BH]
                # sin(phi) = Sin(chi + pi/4) ; cos(phi) = Sin(-chi + pi/4)
                nc.scalar.activation(sin, ph[:], AF.Sin, bias=pi4[:],
                                     scale=1.0)
                nc.scalar.activation(cos, ph[:], AF.Sin, bias=pi4[:],
                                     scale=-1.0)
                if it < N_STEPS:
                    # coupling: vu = [V | U] = dtK @ [sin | cos]
                    nc.tensor.matmul(vu[h][:, 0:BH], kt_sb[:], sin,
                                     start=True, stop=True)
                    nc.tensor.matmul(vu[h][:, BH:2 * BH], kt_sb[:], cos,
                                     start=True, stop=True)
                # band sums -> stash[batch_part, q*128 + (it-1)*4 + j]
                if it > 0:
                    for q in range(BH // P):
                        qg = h * (BH // P) + q
                        base = qg * P + (it - 1) * 4
                        nc.tensor.matmul(
                            stash[:, base:base + 2],
                            sin[:, q * P:(q + 1) * P], wband_sb[:],
                            start=True, stop=True)
                        nc.tensor.matmul(
                            stash[:, base + 2:base + 4],
                            cos[:, q * P:(q + 1) * P], wband_sb[:],
                            start=True, stop=True)
                # one amp0 matmul per (it, h) PE-idle slot
                if it < N_STEPS and job < len(amp_jobs):
                    c, k = amp_jobs[job]
                    job += 1
                    nc.tensor.matmul(amp_ps[c][:],
                                     wa_all[:, (k * NCH + c) * P:
                                            (k * NCH + c + 1) * P],
                                     xk[k], start=(k == 0),
                                     stop=(k == KD - 1))
                    if k == KD - 1:
                        ab = work.tile([P, BL], f32, tag=f"ab{c}",
                                       name=f"ab{c}")
                        nc.scalar.copy(ab[:], amp_ps[c][:])
                        nc.sync.dma_start(
                            amp0_out[:, c * BL:(c + 1) * BL], ab[:])

            def update(h, prio=0):
                """mm + d + wrap for stream h's most recent trig."""
                ph = phi[h]
                cs = cs_live[h]
                with tc.high_priority(offset=prio) if prio else _nullctx():
                    # mm = [cos*V | sin*U] on DVE (Pool cannot read PSUM)
                    mm = work.tile([P, 2 * BH], bf16, tag=f"mm{h}",
                                   name=f"mm{h}")
                    nc.vector.tensor_tensor(mm[:], cs[:], vu[h][:], ALU.mult)
                    # d = sin*U - cos*V on Pool (GPSIMD): unloads DVE so the
                    # other stream's mm can run in this slot
                    d = work.tile([P, BH], bf16, tag=f"d{h}", name=f"d{h}")
                    nc.gpsimd.tensor_tensor(d[:], mm[:, BH:2 * BH],
                                            mm[:, 0:BH], ALU.subtract)
                    # chi' = wrap((chi - d) + dt*omega)
                    nc.vector._custom_dve(wrap_sub, out=ph[:], in0=ph[:],
                                          in1=d[:], s0=dtw_sb, s1=PI,
                                          imm2=TWO_PI)

            # Software-pipelined anti-phase: stream B runs half a step
            # behind A, so B's DVE burst fills A's trig/PE window. The
            # tile_wait_until hints pace the scheduler's model (the runtime
            # ignores them) so the static order interleaves anti-phase.
            def slot(ns):
                return tc.tile_wait_until(ns * 1e-6, enable=pace_ns > 0)

            for it in range(N_STEPS + 1):
                t0 = pace_t0 + it * pace_ns
                with slot(t0):
                    trig_and_pe(0, it)
                if it > 0:
                    with slot(t0):
                        update(1, prio=upd_prio)   # B's step it-1
                with slot(t0 + pace_ns // 2):
                    trig_and_pe(1, it)
                if it < N_STEPS:
                    with slot(t0 + pace_ns // 2):
                        update(0, prio=upd_prio)   # A's step it

            # flush remaining amp jobs (if any) and the stash
            while job < len(amp_jobs):
                c, k = amp_jobs[job]
                job += 1
                nc.tensor.matmul(amp_ps[c][:],
                                 wa_all[:, (k * NCH + c) * P:
                                        (k * NCH + c + 1) * P],
                                 xk[k], start=(k == 0), stop=(k == KD - 1))
                if k == KD - 1:
                    ab = work.tile([P, BL], f32, tag=f"ab{c}", name=f"ab{c}")
                    nc.scalar.copy(ab[:], amp_ps[c][:])
                    nc.sync.dma_start(amp0_out[:, c * BL:(c + 1) * BL], ab[:])
            st_sb = work.tile([P, 4 * N_STEPS * 4], f32, tag="st_sb",
                              name="st_sb")
            nc.scalar.copy(st_sb[:], stash[:])
            nc.sync.dma_start(bs_out[:], st_sb[:])

    nc.compile()
    return nc


def kernel(x, W_phase, W_amp, omega, K):
    from concourse.bass_utils import run_bass_kernel_spmd

    x = np.asarray(x, dtype=np.float32)
    W_phase = np.asarray(W_phase, dtype=np.float32)
    W_amp = np.asarray(W_amp, dtype=np.float32)
    omega = np.asarray(omega, dtype=np.float32)
    K = np.asarray(K, dtype=np.float32)

    # ---- host-side packing (bf16, partition-major: [P, KD*...]) ----
    import ml_dtypes

    def pack_pkm(a_t):
        """[N_DIMS, M] f32 -> [P, KD*M] bf16 with col k*M+j = a_t[k*128+p, j]."""
        kd, m = N_DIMS // P, a_t.shape[1]
        return np.ascontiguousarray(
            a_t.reshape(kd, P, m).transpose(1, 0, 2).reshape(P, kd * m)
        ).astype(ml_dtypes.bfloat16)

    wpT_f = np.zeros((N_DIMS, P), dtype=np.float32)
    wpT_f[:, :ND] = W_phase[:ND].T
    wpT = pack_pkm(wpT_f)
    waT_f = np.zeros((N_DIMS, NCH * P), dtype=np.float32)
    for c in range(NCH):
        n = min(P, N_TOTAL - c * P)
        waT_f[:, c * P:c * P + n] = W_amp[c * P:c * P + n].T
    waT = pack_pkm(waT_f)

    consts = np.zeros((P, P + 3), dtype=np.float32)
    consts[:ND, :ND] = DT * K[:ND, :ND].T
    consts[:N_DELTA, P] = 1.0
    consts[N_DELTA:ND, P + 1] = 1.0
    w = DT * omega[:ND].astype(np.float64)
    consts[:ND, P + 2] = (np.mod(w + PI, TWO_PI) - PI).astype(np.float32)

    if "prog" not in _COMPILED:
        _COMPILED["prog"] = _build_program()
    nc = _COMPILED["prog"]

    in_maps = []
    for i in range(N_CORES):
        xst = pack_pkm(np.ascontiguousarray(x[i * BL:(i + 1) * BL].T))
        in_maps.append({
            "xT": xst, "wpT": wpT, "waT": waT, "consts": consts,
        })

    res = run_bass_kernel_spmd(nc, in_maps, core_ids=list(range(N_CORES)))

    # ---- host-side unshard + exact amp reconstruction ----
    band_of = np.zeros(N_TOTAL, dtype=np.int64)
    band_of[N_DELTA:ND] = 1
    band_of[ND:] = 2

    out = np.empty((BATCH, N_TOTAL), dtype=np.float32)
    for i in range(N_CORES):
        r = res.results[i]
        a0 = np.empty((BL, N_TOTAL))
        raw = r["amp0"].astype(np.float64)          # [128, 3*512]
        for c in range(NCH):
            n = min(P, N_TOTAL - c * P)
            a0[:, c * P:c * P + n] = raw[:n, c * BL:(c + 1) * BL].T
        a0 = np.maximum(np.abs(a0), EPS)

        bs = r["bsums"].astype(np.float64).reshape(P, 4, N_STEPS, 4)
        # [p, q, k, j] -> batch b = q*128+p
        S = np.empty((BL, N_STEPS, 2))
        C = np.empty((BL, N_STEPS, 2))
        for q in range(4):
            sl = slice(q * P, (q + 1) * P)
            S[sl] = bs[:, q, :, 0:2]
            C[sl] = bs[:, q, :, 2:4]
        cosm = C / np.sqrt(S * S + C * C)           # [b, k, band(d,t)]
        f = 1.0 + DT * PAC * cosm
        Pk = np.cumprod(f, axis=1)
        mk = np.minimum.accumulate(Pk, axis=1)
        Pn = Pk[:, -1]                              # [b, 2]
        mn = mk[:, -1]
        Pfac = np.ones((BL, 3))
        Efac = np.ones((BL, 3))
        Pfac[:, 1] = Pn[:, 0]
        Pfac[:, 2] = Pn[:, 1]
        Efac[:, 1] = Pn[:, 0] / mn[:, 0]
        Efac[:, 2] = Pn[:, 1] / mn[:, 1]
        amp = np.maximum(a0 * Pfac[:, band_of], EPS * Efac[:, band_of])
        out[i * BL:(i + 1) * BL] = amp.astype(np.float32)
    return out



# revision 23
# speedup vs baseline: 1.2281x; 1.1538x over previous
"""Trainium2 Bass kernel for DiscreteDeltaThetaGammaLayer.

Coupled Kuramoto-oscillator recurrence:
  phase0 = (x @ W_phase.T) mod 2pi ; amp0 = max(|x @ W_amp.T|, eps)
  32 steps of: intra-band Kuramoto coupling (phase), PAC amplitude modulation
  output: final amp  (4096, 352) f32

Key structural facts exploited:
  - amp never feeds back into phase, K is block-diagonal, and the PAC
    modulation uses only delta/theta band means -> the 256 gamma phases
    never influence the output. Only the 96 delta+theta oscillators need
    the on-device recurrence; amp0 is needed for all 352.
  - K is uniform within each band, so the device only needs per-batch
    band sums (Sd,St,Cd,Ct) per step; the host reconstructs the exact
    clamped amp recurrence in closed form from those.

Device strategy (8 NeuronCores, data-parallel over batch, 512 rows/core):
  - Phase state chi = phi - pi/4 wrapped to [-pi, pi]; sin phi and cos phi
    are then BOTH direct ACT Sin calls (bias=pi/4, scale=+/-1) with args in
    [-1.25pi, 1.25pi] (Sin LUT error <= 2.5e-3 in the outer 12.5% tail).
    No Abs pass.
  - sin/cos written as one bf16 tile [cos | sin]; coupling = 2 bf16 matmuls
    per stream into PSUM [V|U]; mm = cs*vu one TT pass; d = m2-m1 (bf16 2x);
    chi' = WRAP_SUB(chi, d, dt*omega) custom DVE op.
  - Two batch streams (256 each) pipeline the ACT->PE->DVE/Pool chain;
    stream B's mm runs on the Pool engine to unload DVE.
  - Band sums are matmul'd directly into a PSUM stash (col = q*128 +
    step*4 + {Sd,St,Cd,Ct}) and DMA'd once at the end. amp0 f32r matmuls
    are hand-interleaved one-per-step into the PE's idle time and DMA'd
    straight from PSUM; |.| and the clamp run on the host.
"""

import math
import sys

sys.path.insert(0, "/opt/trn_rl_repo")

import numpy as np

# ---- problem constants (module hyperparameters) ----
N_DELTA, N_THETA, N_GAMMA = 32, 64, 256
N_TOTAL = 352
N_DIMS = 1024
BATCH = 4096
N_STEPS = 32
DT = 0.01
COUPLING = 2.0
PAC = 0.3
EPS = 1e-6
TWO_PI = 2.0 * math.pi
PI = math.pi

N_CORES = 8
BL = BATCH // N_CORES          # 512 batch rows per core
NS = 2                         # streams
BH = BL // NS                  # 256 batch per stream
ND = 96                        # delta+theta oscillators on device
P = 128
KD = N_DIMS // P               # 8 contraction chunks
NCH = 3                        # amp0 oscillator chunks (3*128 = 384 >= 352)

LAST_EXEC_NS = None
_COMPILED = {}
_WRAP_SUB = None


def _get_wrap_sub():
    """Custom DVE op: out = wrap((in0 - in1) + s0) into [-s1, s1], period imm2."""
    global _WRAP_SUB
    if _WRAP_SUB is not None:
        return _WRAP_SUB
    from concourse.dve_spec import C0, C1, C2, Spec, Src0, Src1, lower
    from concourse.dve_uop import DveOpSpec
    import concourse.dve_ops as dvo

    def _ref(in0, in1, s0, s1, imm2):
        y = (in0 - in1) + s0
        return (y + imm2 * ((y < -s1).astype(np.float32)
                            - (y > s1).astype(np.float32))).astype(np.float32)

    _y = (Src0 - Src1) + C0
    spec = Spec(body=_y + C2 * ((_y < -C1) - (_y > C1)), reference=_ref)
    shas = {}
    for ver in ("v3", "v4"):
        tmp = DveOpSpec(name="WRAP_SUB_KERNEL", opcode=31,
                        uops=lower(spec, ver=ver), rd1_en=True)
        shas[ver] = tmp.sha(ver)
    op = dvo.DveOp("WRAP_SUB_KERNEL", spec, subdim=False, uops_sha=shas)
    dvo.OPS.append(op)
    dvo.CUSTOM_DVE_SPECS[op.name] = op.spec
    dvo._SUB_OPCODE_FOR_NAME[op.name] = dvo._CUSTOM_DVE_ROW_BASE + len(dvo.OPS) - 1
    _WRAP_SUB = op
    return op


import contextlib


def _nullctx():
    return contextlib.nullcontext()


def _build_program(d_pool=False, split_mm=False, pace_ns=3800, pace_t0=8000,
                   pace_b=1800, pace_u=1400):
    D_POOL, SPLIT_MM = d_pool, split_mm
    import concourse.bass as bass
    import concourse.tile as tile
    from concourse import bacc, mybir

    wrap_sub = _get_wrap_sub()

    f32 = mybir.dt.float32
    f32r = mybir.dt.float32r
    bf16 = mybir.dt.bfloat16
    AF = mybir.ActivationFunctionType
    ALU = mybir.AluOpType

    nc = bacc.Bacc("TRN2", target_bir_lowering=False, debug=False)

    # ---- DRAM I/O ----
    # bf16 inputs, host-packed so partition p's row is contiguous:
    #   xT  [P, KD*BL]  col k*BL+b  = x[b, k*128+p]
    #   wpT [P, KD*P]   col k*P+i   = W_phase[i, k*128+p] (i < ND)
    #   waT [P, KD*NCH*P] col k*NCH*P+c*P+i = W_amp[c*128+i, k*128+p]
    # consts [P, P+3] = [dt*K.T | wband(2) | dtw]
    xT = nc.dram_tensor("xT", [P, KD * BL], bf16, kind="ExternalInput").ap()
    wpT = nc.dram_tensor("wpT", [P, KD * P], bf16, kind="ExternalInput").ap()
    waT = nc.dram_tensor("waT", [P, KD * NCH * P], bf16,
                         kind="ExternalInput").ap()
    consts = nc.dram_tensor("consts", [P, P + 3], f32, kind="ExternalInput").ap()

    amp0_out = nc.dram_tensor("amp0", [P, NCH * BL], f32, kind="ExternalOutput").ap()
    bs_out = nc.dram_tensor("bsums", [P, 4 * N_STEPS * 4], f32,
                            kind="ExternalOutput").ap()
    # bsums col = q*128 + step*4 + {Sd,St,Cd,Ct}; partition = batch q*128+p.

    with tile.TileContext(nc) as tc:
        with (
            tc.tile_pool(name="state", bufs=1) as state_pool,
            tc.tile_pool(name="weights", bufs=1) as wpool,
            tc.tile_pool(name="work", bufs=3) as work,
            tc.tile_pool(name="psum", bufs=1, space="PSUM") as psum,
        ):
    # ---- persistent constants + big packed input loads ----
            cst_sb = wpool.tile([P, P + 3], f32, tag="cst", name="cst_sb")
            nc.sync.dma_start(cst_sb[:], consts[:])
            dtw_sb = cst_sb[:, P + 2:P + 3]
            pi4 = wpool.tile([P, 1], f32, tag="pi4", name="pi4")
            nc.vector.memset(pi4[:], PI / 4.0)
            kt_sb = wpool.tile([P, P], bf16, tag="kt", name="kt_sb")
            nc.vector.tensor_copy(kt_sb[:], cst_sb[:, 0:P])
            wband_sb = wpool.tile([P, 2], bf16, tag="wband", name="wband_sb")
            nc.vector.tensor_copy(wband_sb[:], cst_sb[:, P:P + 2])

            # big packed loads: wp first (proj stationaries), then x in two
            # halves (proj k-chunks start as each half lands), wa last on the
            # gpsimd queue (only gates the lagging amp matmuls).
            wp_all = wpool.tile([P, KD * P], bf16, tag="wp", name="wp_all")
            nc.sync.dma_start(wp_all[:], wpT[:])
            QK = KD // 4
            x_t = []
            for q in range(4):
                t = wpool.tile([P, QK * BL], bf16, tag=f"xq{q}",
                               name=f"x_q{q}")
                nc.sync.dma_start(t[:], xT[:, q * QK * BL:(q + 1) * QK * BL])
                x_t.append(t)
            # wa last on the same (sync) queue: transfers stay behind x on
            # the shared DMA engines; it only gates the lagging amp matmuls.
            wa_all = wpool.tile([P, KD * NCH * P], bf16, tag="wa",
                                name="wa_all")
            nc.sync.dma_start(wa_all[:], waT[:])
            wpk = [wp_all[:, k * P:(k + 1) * P] for k in range(KD)]

            def x_sl(k, lo, hi):
                t = x_t[k // QK]
                kk = k % QK
                return t[:, kk * BL + lo:kk * BL + hi]

            xk = [x_sl(k, 0, BL) for k in range(KD)]

            # ---- PSUM tiles ----
            vu = [psum.tile([P, 2 * BH], f32, tag=f"vu{h}", name=f"vu{h}")
                  for h in range(NS)]
            stash = psum.tile([P, 4 * N_STEPS * 4], f32, tag="stash",
                              name="stash")
            amp_ps = [psum.tile([P, BL], f32, tag=f"amp{c}", name=f"amp{c}")
                      for c in range(NCH)]

            # ---- phase0 projection (f32r, 256-wide => full PE rate) ----
            phi = [state_pool.tile([P, BH], f32, tag=f"phi{h}", name=f"phi{h}")
                   for h in range(NS)]
            for h in range(NS):
                dst = vu[h][:, 0:BH]
                for k in range(KD):
                    nc.tensor.matmul(dst, wpk[k],
                                     x_sl(k, h * BH, (h + 1) * BH),
                                     start=(k == 0), stop=(k == KD - 1))
                # chi0 = wrap(phase0 - pi/4)
                nc.vector.add_range_wrap(phi[h][:], dst, -PI / 4.0, PI,
                                         TWO_PI)

            # ---- recurrence ----
            # amp0 matmuls are drip-fed into PE idle slots: one per (it, h).
            amp_jobs = [(c, k) for c in range(NCH) for k in range(KD)]
            job = 0

            cs_live = {}

            def trig(h, it):
                """sin/cos + coupling matmuls for (h, it)."""
                ph = phi[h]
                cs = work.tile([P, 2 * BH], bf16, tag=f"cs{h}", name=f"cs{h}")
                cs_live[h] = cs
                cos = cs[:, 0:BH]
                sin = cs[:, BH:2 * BH]
                # sin(phi) = Sin(chi + pi/4) ; cos(phi) = Sin(-chi + pi/4)
                nc.scalar.activation(sin, ph[:], AF.Sin, bias=pi4[:],
                                     scale=1.0)
                nc.scalar.activation(cos, ph[:], AF.Sin, bias=pi4[:],
                                     scale=-1.0)
                if it < N_STEPS:
                    # coupling: vu = [V | U] = dtK @ [sin | cos]
                    nc.tensor.matmul(vu[h][:, 0:BH], kt_sb[:], sin,
                                     start=True, stop=True)
                    nc.tensor.matmul(vu[h][:, BH:2 * BH], kt_sb[:], cos,
                                     start=True, stop=True)

            def bands(h, it):
                """band sums -> stash[batch_part, (it-1)*16 + qg*4 + j]

                Step-major columns so steps 0..27 flush early as one
                contiguous copy+DMA and only the last 4 steps sit on the
                tail."""
                cs = cs_live[h]
                cos = cs[:, 0:BH]
                sin = cs[:, BH:2 * BH]
                for q in range(BH // P):
                    qg = h * (BH // P) + q
                    base = (it - 1) * 16 + qg * 4
                    nc.tensor.matmul(
                        stash[:, base:base + 2],
                        sin[:, q * P:(q + 1) * P], wband_sb[:],
                        start=True, stop=True)
                    nc.tensor.matmul(
                        stash[:, base + 2:base + 4],
                        cos[:, q * P:(q + 1) * P], wband_sb[:],
                        start=True, stop=True)

            def update(h, d_pool=True, split_mm=False):
                """mm + d + wrap for stream h's most recent trig."""
                ph = phi[h]
                cs = cs_live[h]
                # mm = [cos*V | sin*U] on DVE (only DVE can read PSUM)
                mm = work.tile([P, 2 * BH], bf16, tag=f"mm{h}",
                               name=f"mm{h}")
                if split_mm:
                    nc.vector.tensor_tensor(mm[:, 0:BH], cs[:, 0:BH],
                                            vu[h][:, 0:BH], ALU.mult)
                    nc.vector.tensor_tensor(mm[:, BH:2 * BH],
                                            cs[:, BH:2 * BH],
                                            vu[h][:, BH:2 * BH], ALU.mult)
                else:
                    nc.vector.tensor_tensor(mm[:], cs[:], vu[h][:], ALU.mult)
                # d = sin*U - cos*V
                d = work.tile([P, BH], bf16, tag=f"d{h}", name=f"d{h}")
                eng = nc.gpsimd if d_pool else nc.vector
                eng.tensor_tensor(d[:], mm[:, BH:2 * BH], mm[:, 0:BH],
                                  ALU.subtract)
                # chi' = wrap((chi - d) + dt*omega)
                nc.vector._custom_dve(wrap_sub, out=ph[:], in0=ph[:],
                                      in1=d[:], s0=dtw_sb, s1=PI,
                                      imm2=TWO_PI)

            def amp_drip(n):
                """emit n amp0 matmul jobs; copy+DMA when a chunk completes."""
                nonlocal job
                for _ in range(n):
                    if job >= len(amp_jobs):
                        return
                    c, k = amp_jobs[job]
                    job += 1
                    nc.tensor.matmul(amp_ps[c][:],
                                     wa_all[:, (k * NCH + c) * P:
                                            (k * NCH + c + 1) * P],
                                     xk[k], start=(k == 0),
                                     stop=(k == KD - 1))
                    if k == KD - 1:
                        ab = work.tile([P, BL], f32, tag=f"ab{c}",
                                       name=f"ab{c}")
                        nc.scalar.copy(ab[:], amp_ps[c][:])
                        nc.sync.dma_start(
                            amp0_out[:, c * BL:(c + 1) * BL], ab[:])

            # pacing hints steer the (greedy, sim-driven) tile scheduler.
            # The hint is a floor in the scheduler's VIRTUAL timeline; the
            # realized order per engine follows hint order (ties broken by
            # emission order), so these fix the per-engine static order:
            #   ACT: sin_A cos_A .. sin_B cos_B ; DVE: [mm d wrap]_A then _B
            def slot(ns):
                return tc.tile_wait_until(ns * 1e-6, enable=pace_ns > 0)

            for it in range(N_STEPS + 1):
                base = pace_t0 + it * pace_ns
                with slot(base):
                    trig(0, it)
                with slot(base + pace_b):
                    trig(1, it)
                if it > 0:
                    bands(0, it)
                    bands(1, it)
                if it < N_STEPS:
                    with slot(base + pace_u):
                        update(0, d_pool=D_POOL, split_mm=SPLIT_MM)
                    with slot(base + pace_b + pace_u):
                        update(1, d_pool=D_POOL, split_mm=SPLIT_MM)
                    with slot(base + 2800):
                        amp_drip(1)
                if it == N_STEPS - 3:
                    # steps 0..27 are complete in the stash: flush them now
                    # so only the last 4 steps' 64 cols sit on the tail.
                    with slot(base + 2800):
                        st_e = work.tile([P, 28 * 16], f32, tag="ste",
                                        name="st_early")
                        nc.scalar.copy(st_e[:], stash[:, 0:28 * 16])
                        nc.sync.dma_start(bs_out[:, 0:28 * 16], st_e[:])

            # flush remaining amp jobs (if any) and the stash tail
            amp_drip(len(amp_jobs))
            st_sb = work.tile([P, 4 * 16], f32, tag="st_sb", name="st_sb")
            nc.scalar.copy(st_sb[:], stash[:, 28 * 16:32 * 16])
            nc.sync.dma_start(bs_out[:, 28 * 16:32 * 16], st_sb[:])

    nc.compile()
    return nc


def kernel(x, W_phase, W_amp, omega, K):
    from concourse.bass_utils import run_bass_kernel_spmd

    x = np.asarray(x, dtype=np.float32)
    W_phase = np.asarray(W_phase, dtype=np.float32)
    W_amp = np.asarray(W_amp, dtype=np.float32)
    omega = np.asarray(omega, dtype=np.float32)
    K = np.asarray(K, dtype=np.float32)

    # ---- host-side packing (bf16, partition-major: [P, KD*...]) ----
    import ml_dtypes

    def pack_pkm(a_t):
        """[N_DIMS, M] f32 -> [P, KD*M] bf16 with col k*M+j = a_t[k*128+p, j]."""
        kd, m = N_DIMS // P, a_t.shape[1]
        return np.ascontiguousarray(
            a_t.reshape(kd, P, m).transpose(1, 0, 2).reshape(P, kd * m)
        ).astype(ml_dtypes.bfloat16)

    wpT_f = np.zeros((N_DIMS, P), dtype=np.float32)
    wpT_f[:, :ND] = W_phase[:ND].T
    wpT = pack_pkm(wpT_f)
    waT_f = np.zeros((N_DIMS, NCH * P), dtype=np.float32)
    for c in range(NCH):
        n = min(P, N_TOTAL - c * P)
        waT_f[:, c * P:c * P + n] = W_amp[c * P:c * P + n].T
    waT = pack_pkm(waT_f)

    consts = np.zeros((P, P + 3), dtype=np.float32)
    consts[:ND, :ND] = DT * K[:ND, :ND].T
    consts[:N_DELTA, P] = 1.0
    consts[N_DELTA:ND, P + 1] = 1.0
    w = DT * omega[:ND].astype(np.float64)
    consts[:ND, P + 2] = (np.mod(w + PI, TWO_PI) - PI).astype(np.float32)

    if "prog" not in _COMPILED:
        _COMPILED["prog"] = _build_program()
    nc = _COMPILED["prog"]

    in_maps = []
    for i in range(N_CORES):
        xst = pack_pkm(np.ascontiguousarray(x[i * BL:(i + 1) * BL].T))
        in_maps.append({
            "xT": xst, "wpT": wpT, "waT": waT, "consts": consts,
        })

    res = run_bass_kernel_spmd(nc, in_maps, core_ids=list(range(N_CORES)))

    # ---- host-side unshard + exact amp reconstruction ----
    band_of = np.zeros(N_TOTAL, dtype=np.int64)
    band_of[N_DELTA:ND] = 1
    band_of[ND:] = 2

    out = np.empty((BATCH, N_TOTAL), dtype=np.float32)
    for i in range(N_CORES):
        r = res.results[i]
        a0 = np.empty((BL, N_TOTAL))
        raw = r["amp0"].astype(np.float64)          # [128, 3*512]
        for c in range(NCH):
            n = min(P, N_TOTAL - c * P)
            a0[:, c * P:c * P + n] = raw[:n, c * BL:(c + 1) * BL].T
        a0 = np.maximum(np.abs(a0), EPS)

        bs = r["bsums"].astype(np.float64).reshape(P, N_STEPS, 4, 4)
        # [p, k, q, j] -> batch b = q*128+p (step-major stash columns)
        S = np.empty((BL, N_STEPS, 2))
        C = np.empty((BL, N_STEPS, 2))
        for q in range(4):
            sl = slice(q * P, (q + 1) * P)
            S[sl] = bs[:, :, q, 0:2]
            C[sl] = bs[:, :, q, 2:4]
        cosm = C / np.sqrt(S * S + C * C)           # [b, k, band(d,t)]
        f = 1.0 + DT * PAC * cosm
        Pk = np.cumprod(f, axis=1)
        mk = np.minimum.accumulate(Pk, axis=1)
        Pn = Pk[:, -1]                              # [b, 2]
        mn = mk[:, -1]
        Pfac = np.ones((BL, 3))
        Efac = np.ones((BL, 3))
        Pfac[:, 1] = Pn[:, 0]
        Pfac[:, 2] = Pn[:, 1]
        Efac[:, 1] = Pn[:, 0] / mn[:, 0]
        Efac[:, 2] = Pn[:, 1] / mn[:, 1]
        amp = np.maximum(a0 * Pfac[:, band_of], EPS * Efac[:, band_of])
        out[i * BL:(i + 1) * BL] = amp.astype(np.float32)
    return out

